# revision 8
# baseline (speedup 1.0000x reference)
"""GCN encoder (3-layer) Trainium2 kernel, 8-core SPMD.

Strategy (v2):
  Per layer l: out = relu((A @ T_l) @ W_l + b_l), with T_0 = X and
  T_{l+1} = out_l (post-ReLU activations, 128 wide), A = normalized adjacency
  (fixed across layers).  Aggregate-then-transform: the dense W_l is applied
  AFTER aggregation (associativity), so gathered table rows are raw
  activations.
  - Nodes sharded by contiguous id range across 8 cores (dst partition).
  - Layer 0 needs NO gather and NO allgather: its table is the INPUT X, so the
    host pre-expands X into dst-major slot order (per core) and the kernel
    streams slot blocks with plain DMAs.
  - Layers 1-2: AllGather of the [N,128] bf16 activation table, then per-core
    bulk dma_gather of source rows (gpsimd SWDGE - measured ~8.6 ns/idx, the
    dominant cost) + one-hot matmul segment-sum into PSUM ([feat, dst]
    transposed), then per-window dense + bias + relu.
  - The scaled one-hot blocks (norm values scattered into [128, blk, WIN]) are
    precomputed on host and streamed; identical for all 3 layers.
All graph structure (window caps, block offsets) is computed from the inputs
at call time and baked into the compiled program; caps are maxed across cores
so all 8 cores run one SPMD program.
"""

import math
import os
import numpy as np
from contextlib import ExitStack

from ml_dtypes import bfloat16

import concourse.bass as bass
import concourse.bacc as bacc
import concourse.mybir as mybir
import concourse.tile as tile
from concourse.bass_utils import run_bass_kernel_spmd
from concourse import library_config

F32 = mybir.dt.float32
BF16 = mybir.dt.bfloat16
I16 = mybir.dt.int16

NCORES = 8
D = 128          # feature width (layers 0/1/2 input, gather row width)
DOUT = 64        # final output width
WIN = 48         # dst nodes per window (one-hot width, psum free dim)
NCHUNK = 4       # src chunks (int16 gather index range)
GRP = 16         # windows per gather-call group


class Plan:
    """Host-derived, core-invariant schedule + per-core tensor data."""

    def __init__(self, n_nodes, edge_src, edge_dst, edge_norm):
        self.N = n_nodes
        self.WPC = math.ceil(n_nodes / (NCORES * WIN))   # windows per core
        self.SH = self.WPC * WIN                         # table rows per core
        self.NTAB = NCORES * self.SH
        assert self.NTAB % NCHUNK == 0
        self.CHROWS = self.NTAB // NCHUNK
        assert self.CHROWS <= 32768, self.CHROWS
        self.NG = math.ceil(self.WPC / GRP)

        core = edge_dst // self.SH
        wloc = (edge_dst % self.SH) // WIN
        dloc = edge_dst % WIN
        ch = edge_src // self.CHROWS

        # per-(core,window,chunk) counts -> shared caps (multiple of 128)
        cell = (core * self.WPC + wloc) * NCHUNK + ch
        counts = np.bincount(cell, minlength=NCORES * self.WPC * NCHUNK)
        counts = counts.reshape(NCORES, self.WPC, NCHUNK)
        max8 = counts.max(axis=0)
        self.cap = 128 * np.maximum(1, -(-max8 // 128))   # [WPC, NCHUNK]

        # canonical slot/block order: for g: for ch: for w in g: cap[w,ch]
        self.groups = []
        for g in range(self.NG):
            self.groups.append(list(range(g * GRP, min((g + 1) * GRP, self.WPC))))
        slotoff = {}   # (ch, w) -> global slot offset
        self.roff = {}  # (ch, w) -> block offset within its (g,ch) region
        self.rslot = []  # [g][ch] region slot count
        self.icol = []   # [g][ch] idx-table col offset
        off = 0
        for g, ws in enumerate(self.groups):
            rs = []
            ic = []
            for c in range(NCHUNK):
                r0 = off
                for w in ws:
                    slotoff[(c, w)] = off
                    self.roff[(c, w)] = (off - r0) // 128
                    off += int(self.cap[w, c])
                rs.append(off - r0)
                ic.append(r0 // 16)
            self.rslot.append(rs)
            self.icol.append(ic)
        self.TOTSLOT = off
        self.TOTBLK = off // 128
        self.slotoff = slotoff
        # one-hot block index = slot offset // 128 (same canonical order)
        self.ohblk = {k: v // 128 for k, v in slotoff.items()}
        # per-group one-hot col range (flat [TOTBLK*WIN])
        self.gohcol = []
        for g, ws in enumerate(self.groups):
            b0 = self.ohblk[(0, ws[0])]
            nb = sum(int(self.cap[w, c]) for c in range(NCHUNK) for w in ws) // 128
            self.gohcol.append((b0, nb))

        self.edge_core = core
        self.edge_w = wloc
        self.edge_d = dloc
        self.edge_ch = ch
        self.edge_src = edge_src
        self.edge_norm = edge_norm

    def core_slots(self, c):
        """Slot assignment for core c: (slot_ids, global_src, local_d, norm)."""
        sel = np.nonzero(self.edge_core == c)[0]
        w = self.edge_w[sel]
        ch = self.edge_ch[sel]
        d = self.edge_d[sel]
        src = self.edge_src[sel]
        norm = self.edge_norm[sel]
        # rank within (w, ch) bucket
        order = np.lexsort((ch, w))
        w, ch, d, src, norm = w[order], ch[order], d[order], src[order], norm[order]
        bucket = w * NCHUNK + ch
        nb = self.WPC * NCHUNK
        bc = np.bincount(bucket, minlength=nb)
        starts = np.concatenate([[0], np.cumsum(bc)[:-1]])
        rank = np.arange(len(sel)) - starts[bucket]
        base = np.empty(nb, dtype=np.int64)
        for wi in range(self.WPC):
            for ci in range(NCHUNK):
                base[wi * NCHUNK + ci] = self.slotoff[(ci, wi)]
        slot = base[bucket] + rank
        return slot, src, ch, d, norm

    def core_tensors(self, c, x_full):
        """Build idx table, one-hot table and expanded-X slots for core c."""
        slot, src, ch, d, norm = self.core_slots(c)

        idxs = np.zeros(self.TOTSLOT, dtype=np.int16)
        idxs[slot] = (src - ch * self.CHROWS).astype(np.int16)
        # wrapped idx layout: i at [i%16, i//16] per gather call; calls are
        # contiguous ranges whose lengths are multiples of 16, so a global
        # wrap matches the per-call wrap.
        iw = idxs.reshape(-1, 16).T.copy()            # [16, TOTSLOT//16]
        iw = np.tile(iw, (8, 1))                      # replicate to 128 parts

        oh = np.zeros((self.TOTBLK, 128, WIN), dtype=np.float32)
        oh[slot // 128, slot % 128, d] = norm
        oh = oh.transpose(1, 0, 2).reshape(128, self.TOTBLK * WIN)

        # layer-0 slot table: X rows expanded into slot order (zeros at pads)
        xs = np.zeros((self.TOTSLOT, D), dtype=np.float32)
        xs[slot] = x_full[src]
        xs = xs.reshape(self.TOTBLK, 128, D).transpose(1, 0, 2)
        return iw, oh.astype(bfloat16), np.ascontiguousarray(xs).astype(bfloat16)


def _build_nc(plan, enable_asserts=False):
    p = plan
    no_coll = bool(int(os.environ.get("GCN_NO_COLL", "0")))
    nc = bacc.Bacc(
        "TRN2",
        target_bir_lowering=False,
        debug=False,
        enable_asserts=enable_asserts,
        num_devices=NCORES,
    )
    x0s = nc.dram_tensor("x0s", [128, p.TOTBLK, D], BF16, kind="ExternalInput")
    xsh = nc.dram_tensor("xsh", [p.SH, D], BF16, kind="ExternalInput")
    ohs = nc.dram_tensor("ohs", [p.WPC, WIN, WIN], BF16, kind="ExternalInput")
    w0 = nc.dram_tensor("w0", [D, D], BF16, kind="ExternalInput")
    w1 = nc.dram_tensor("w1", [D, D], BF16, kind="ExternalInput")
    w2 = nc.dram_tensor("w2", [D, D], BF16, kind="ExternalInput")
    b0 = nc.dram_tensor("b0", [WIN, D], F32, kind="ExternalInput")
    b1 = nc.dram_tensor("b1", [WIN, D], F32, kind="ExternalInput")
    b2 = nc.dram_tensor("b2", [WIN, D], F32, kind="ExternalInput")
    idxt = nc.dram_tensor("idxt", [128, p.TOTSLOT // 16], I16, kind="ExternalInput")
    oht = nc.dram_tensor("oht", [128, p.TOTBLK * WIN], BF16, kind="ExternalInput")
    outp = nc.dram_tensor("outp", [p.SH, DOUT], F32, kind="ExternalOutput")

    with tile.TileContext(nc) as tc, ExitStack() as ctx:
        nc.gpsimd.load_library(library_config.mlp)
        sb = ctx.enter_context(tc.tile_pool(name="sb", bufs=2))
        sbp = ctx.enter_context(tc.tile_pool(name="sbp", bufs=1))
        ps = ctx.enter_context(tc.tile_pool(name="ps", bufs=2, space="PSUM"))
        dram = ctx.enter_context(tc.tile_pool(name="dram", bufs=1, space="DRAM"))

        t_sha = dram.tile([p.SH, D], BF16)
        t_shb = dram.tile([p.SH, D], BF16)
        t_tabs = [dram.tile([p.NTAB, D], BF16, addr_space="Shared",
                            name=f"t_tab{i}") for i in range(2)]

        # persistent tiles
        idx_sb = sbp.tile([128, p.TOTSLOT // 16], I16)
        nc.sync.dma_start(out=idx_sb[:], in_=idxt[:])
        wt = []
        for wsrc in (w0, w1, w2):
            w_sb = sbp.tile([D, D], BF16, name=f"w_{wsrc.name}")
            nc.sync.dma_start(out=w_sb[:], in_=wsrc[:])
            wt.append(w_sb)
        bt = []
        for bsrc in (b0, b1, b2):
            b_sb = sbp.tile([WIN, D], F32, name=f"b_{bsrc.name}")
            nc.sync.dma_start(out=b_sb[:], in_=bsrc[:])
            bt.append(b_sb)

        def do_allgather(src_tile, dst_tile):
            if no_coll:
                nc.sync.dma_start(out=dst_tile[:p.SH, :], in_=src_tile[:])
                return
            nc.gpsimd.collective_compute(
                "AllGather",
                mybir.AluOpType.bypass,
                replica_groups=[list(range(NCORES))],
                ins=[src_tile.opt()],
                outs=[dst_tile.opt()],
            )

        for layer in range(3):
            t_tab = t_tabs[layer - 1] if layer > 0 else None
            t_out = t_sha if layer == 0 else t_shb
            t_self = [xsh, t_sha, t_shb][layer]
            for g, ws in enumerate(p.groups):
                regs = []
                for c in range(NCHUNK):
                    nslots = p.rslot[g][c]
                    reg = sb.tile([128, nslots // 128, 128], BF16,
                                  tag=f"reg{c}", name=f"reg_{layer}_{g}_{c}")
                    ic0 = p.icol[g][c]
                    if layer == 0:
                        blk0 = ic0 // 8
                        nc.sync.dma_start(
                            out=reg[:],
                            in_=x0s[:, blk0:blk0 + nslots // 128, :])
                    else:
                        nc.gpsimd.dma_gather(
                            out_ap=reg[:],
                            in_ap=t_tab[c * p.CHROWS:(c + 1) * p.CHROWS, :],
                            idxs_ap=idx_sb[:, ic0:ic0 + nslots // 16],
                            num_idxs=nslots,
                            num_idxs_reg=nslots,
                            elem_size=D,
                            single_packet=False,
                        )
                    regs.append(reg)
                b0g, nbg = p.gohcol[g]
                oh_g = sb.tile([128, nbg, WIN], BF16, tag="oh",
                               name=f"oh_{layer}_{g}")
                nc.sync.dma_start(
                    out=oh_g[:], in_=oht[:, b0g * WIN:(b0g + nbg) * WIN])
                for w in ws:
                    r0 = w * WIN
                    # self-loop contribution: 48 consecutive local rows
                    tself = sb.tile([WIN, D], BF16, tag="tself",
                                    name=f"tself_{layer}_{w}")
                    nc.sync.dma_start(out=tself[:], in_=t_self[r0:r0 + WIN, :])
                    ohw = sb.tile([WIN, WIN], BF16, tag="ohs",
                                  name=f"ohs_{layer}_{w}")
                    nc.sync.dma_start(out=ohw[:], in_=ohs[w, :, :])
                    pT = ps.tile([D, WIN], F32, tag="pagg", name=f"pagg_{layer}_{w}")
                    k = 0
                    for c in range(NCHUNK):
                        ro = p.roff[(c, w)]
                        ob = p.ohblk[(c, w)] - b0g
                        for j in range(int(p.cap[w, c]) // 128):
                            nc.tensor.matmul(
                                out=pT[:],
                                lhsT=regs[c][:, ro + j, :],
                                rhs=oh_g[:, ob + j, :],
                                start=(k == 0),
                                stop=False,
                            )
                            k += 1
                    nc.tensor.matmul(out=pT[:], lhsT=tself[:], rhs=ohw[:],
                                     start=False, stop=True)
                    # aggregate-then-transform: u = cast(pT), v = u^T @ W_l
                    u = sb.tile([D, WIN], BF16, tag="u", name=f"u_{layer}_{w}")
                    nc.vector.tensor_copy(out=u[:], in_=pT[:])
                    v = ps.tile([WIN, D], F32, tag="pden", name=f"pden_{layer}_{w}")
                    nc.tensor.matmul(out=v[:], lhsT=u[:], rhs=wt[layer][:],
                                     start=True, stop=True)
                    if layer < 2:
                        tb = sb.tile([WIN, D], F32, tag="tb",
                                     name=f"tb_{layer}_{w}")
                        nc.vector.scalar_tensor_tensor(
                            out=tb[:], in0=v[:], scalar=1.0, in1=bt[layer][:],
                            op0=mybir.AluOpType.mult, op1=mybir.AluOpType.add)
                        tt2 = sb.tile([WIN, D], BF16, tag="tnext",
                                      name=f"tnext_{layer}_{w}")
                        nc.scalar.activation(
                            out=tt2[:], in_=tb[:],
                            func=mybir.ActivationFunctionType.Relu)
                        nc.sync.dma_start(out=t_out[r0:r0 + WIN, :], in_=tt2[:])
                    else:
                        tb = sb.tile([WIN, D], F32, tag="tb", name=f"tb_2_{w}")
                        nc.vector.scalar_tensor_tensor(
                            out=tb[:], in0=v[:], scalar=1.0, in1=bt[2][:],
                            op0=mybir.AluOpType.mult, op1=mybir.AluOpType.add)
                        nc.sync.dma_start(out=outp[r0:r0 + WIN, :],
                                          in_=tb[:, :DOUT])
            if layer < 2:
                do_allgather(t_out, t_tabs[layer])
    nc.compile()
    return nc


def _prep(x, edge_index, edge_weight):
    """Real edges (src, dst, norm) + per-node self-loop norm (1/deg)."""
    N = x.shape[0]
    src = np.asarray(edge_index[0], dtype=np.int64)
    dst = np.asarray(edge_index[1], dtype=np.int64)
    w = np.asarray(edge_weight, dtype=np.float64)
    deg = np.bincount(dst, weights=w, minlength=N) + 1.0  # + self loop
    dis = 1.0 / np.sqrt(deg)
    norm = (dis[src] * w * dis[dst]).astype(np.float32)
    selfnorm = (dis * dis).astype(np.float32)
    return src.astype(np.int64), dst.astype(np.int64), norm, selfnorm


def kernel(x, edge_index, edge_weight, W0, b0, W1, b1, W2, b2):
    x = np.asarray(x, dtype=np.float32)
    N = x.shape[0]
    src_f, dst_f, norm, selfnorm = _prep(x, edge_index, edge_weight)
    plan = Plan(N, src_f, dst_f, norm)

    W2p = np.zeros((D, D), dtype=np.float32)
    W2p[:, :DOUT] = np.asarray(W2)
    b2p = np.zeros(D, dtype=np.float32)
    b2p[:DOUT] = np.asarray(b2)
    bb0 = np.tile(np.asarray(b0, np.float32)[None, :], (WIN, 1))
    bb1 = np.tile(np.asarray(b1, np.float32)[None, :], (WIN, 1))
    bb2 = np.tile(b2p[None, :], (WIN, 1))

    in_maps = []
    for c in range(NCORES):
        iw, oh, x0slots = plan.core_tensors(c, x)
        r0 = c * plan.SH
        real = max(0, min(r0 + plan.SH, N) - r0)
        xs = np.zeros((plan.SH, D), dtype=np.float32)
        sn = np.zeros(plan.SH, dtype=np.float32)
        if real > 0:
            xs[:real] = x[r0:r0 + real]
            sn[:real] = selfnorm[r0:r0 + real]
        ohself = np.zeros((plan.WPC, WIN, WIN), dtype=np.float32)
        ii = np.arange(WIN)
        ohself[:, ii, ii] = sn.reshape(plan.WPC, WIN)
        in_maps.append({
            "x0s": x0slots,
            "xsh": xs.astype(bfloat16),
            "ohs": ohself.astype(bfloat16),
            "w0": np.asarray(W0).astype(bfloat16),
            "w1": np.asarray(W1).astype(bfloat16),
            "w2": W2p.astype(bfloat16),
            "b0": bb0,
            "b1": bb1,
            "b2": bb2,
            "idxt": iw,
            "oht": oh,
        })

    nc = _build_nc(plan)
    trace = bool(int(os.environ.get("GCN_TRACE", "0")))
    if trace:
        _ensure_ntff_hook()
    res = run_bass_kernel_spmd(
        nc, in_maps, list(range(NCORES)),
        trace=trace, tmpdir=os.environ.get("GCN_TRACE_DIR"),
    )
    shards = [res.results[c]["outp"] for c in range(NCORES)]
    out = np.concatenate(shards, axis=0)[:N]
    if res.exec_time_ns is not None:
        kernel.last_exec_time_ns = res.exec_time_ns
    return out.astype(np.float32)


kernel.last_exec_time_ns = None


def _ensure_ntff_hook():
    """Inject the missing antenv.axon_hooks shim + local artifact stash so
    run_bass_kernel_spmd(trace=True) can capture NTFF profiles under axon."""
    import sys
    import types
    import concourse.bass_utils as bu
    if "antenv.axon_hooks" not in sys.modules:
        mod = types.ModuleType("antenv.axon_hooks")
        mod._hook = None

        def set_axon_ntff_profile_hook(h):
            mod._hook = h

        def get_axon_ntff_profile_hook():
            return mod._hook

        mod.set_axon_ntff_profile_hook = set_axon_ntff_profile_hook
        mod.get_axon_ntff_profile_hook = get_axon_ntff_profile_hook
        sys.modules["antenv.axon_hooks"] = mod
        try:
            from trn_agent_boot.trn_boot import _ntff_profile_via_ctypes
            mod._hook = _ntff_profile_via_ctypes("/opt/axon/libaxon_pjrt.so")
        except Exception as e:
            print("ntff hook setup failed:", e)
    bu.upload_artifacts = lambda tmpdir: f"local:{tmpdir}"
